# revision 3
# baseline (speedup 1.0000x reference)
"""Trainium2 Bass kernel for nn_MultiHeadAttention (B=4, S=2048, D=512, H=8).

Sharding: tensor-parallel over heads — core c owns head c (Dh=64).
Each core computes q/k/v projections for its head slice (full x replicated,
host-pre-transposed to x^T in bf16), attention for its head over all 4
batches, and the partial out-projection O_c @ Wo[c]; the host divides each
partial by its softmax denominator (returned separately) and sums the 8
partials (the TP all-reduce done at gather time), adding the biases that
commute with that reduction (bo, bv@Wo).

Engine plan: ACT does exclusively exp (128 x [128,1024] instrs, ~1.15us each
— the pacing engine); every PE matmul is shaped to co-stream in pairs on
disjoint 64-row or 64-column groups of the 128x128 array:
  - S^T: 64-contraction row-group pairs (batch even/odd).
  - AV: 64-wide V weights at column groups 0/64 (batch pair), writing the
    two halves of one po[128,512] bank; softmax denominators ride as
    separate 1-col ones-weight matmuls paired with the opposite batch's AV.
  - out-proj: 64-contraction row-group pairs (Wo duplicated on both
    partition halves).
  - q/k/v prep: 64-wide weight column-group pairs (batch even/odd).
Batches are paired [even; odd] on SBUF partition halves. Prep/transpose/
out-proj work is drip-fed between attention tiles from filler queues so the
PE never delays the next exp.
"""
import numpy as np

import concourse.bass as bass
import concourse.mybir as mybir
import concourse.tile as tile
from concourse import bacc
from concourse.bass_utils import run_bass_kernel_spmd

B, S, D = 4, 2048, 512
H, DH = 8, 64
NCORES = 8
F32 = mybir.dt.float32
BF16 = mybir.dt.bfloat16
AF = mybir.ActivationFunctionType

NKT = S // 128          # 16 key tiles per batch
NQB = S // 512          # 4 query blocks per batch
NCH = D // 128          # 4 d_model chunks

_NC_CACHE = {}


def build_kernel():
    nc = bacc.Bacc("TRN2", target_bir_lowering=False, debug=False)

    xT = nc.dram_tensor("xT", [B, D, S], BF16, kind="ExternalInput")
    wq = nc.dram_tensor("wq", [D, DH], BF16, kind="ExternalInput")
    wk = nc.dram_tensor("wk", [D, DH], BF16, kind="ExternalInput")
    wv = nc.dram_tensor("wv", [D, DH], BF16, kind="ExternalInput")
    wo_dup = nc.dram_tensor("wo_dup", [128, D], BF16, kind="ExternalInput")
    bq = nc.dram_tensor("bq", [128, 1], F32, kind="ExternalInput")
    bk = nc.dram_tensor("bk", [128, 1], F32, kind="ExternalInput")
    idin = nc.dram_tensor("idin", [128, 128], BF16, kind="ExternalInput")
    onesw = nc.dram_tensor("onesw", [128, 1], BF16, kind="ExternalInput")
    out = nc.dram_tensor("out", [B * S, D], BF16, kind="ExternalOutput")
    den = nc.dram_tensor("den", [B, S], F32, kind="ExternalOutput")

    with tile.TileContext(nc) as tc:
        with (
            tc.tile_pool(name="consts", bufs=1) as consts,
            tc.tile_pool(name="xtp", bufs=16) as xtp,
            tc.tile_pool(name="qkp", bufs=2) as qkp,
            tc.tile_pool(name="vtp", bufs=2) as vtp,
            tc.tile_pool(name="vp", bufs=4) as vp,
            tc.tile_pool(name="ptp", bufs=3) as ptp,
            tc.tile_pool(name="otp", bufs=2) as otp,
            tc.tile_pool(name="dnp", bufs=2) as dnp,
            tc.tile_pool(name="outp", bufs=4) as outp,
            tc.tile_pool(name="psA", bufs=2, space="PSUM") as psA,   # pst [128,1024] x2 = 4 banks
            tc.tile_pool(name="psO", bufs=1, space="PSUM") as psO,   # po  [128,512]     1 bank
            tc.tile_pool(name="psD", bufs=1, space="PSUM") as psDp,  # den [128,512]     1 bank
            tc.tile_pool(name="psM", bufs=2, space="PSUM") as psM,   # misc [128,512] x2 2 banks
        ):
            wq_sb = consts.tile([128, NCH, DH], BF16)
            wk_sb = consts.tile([128, NCH, DH], BF16)
            wv_sb = consts.tile([128, NCH, DH], BF16)
            wo_sb = consts.tile([128, D], BF16)
            bq_sb = consts.tile([128, 1], F32)
            bk_sb = consts.tile([128, 1], F32)
            ident = consts.tile([128, 128], BF16)
            ones_sb = consts.tile([128, 1], BF16)
            nc.sync.dma_start(out=wq_sb[:], in_=wq.rearrange("(c p) m -> p c m", p=128))
            nc.sync.dma_start(out=wk_sb[:], in_=wk.rearrange("(c p) m -> p c m", p=128))
            nc.sync.dma_start(out=wv_sb[:], in_=wv.rearrange("(c p) m -> p c m", p=128))
            nc.sync.dma_start(out=wo_sb[:], in_=wo_dup[:])
            nc.sync.dma_start(out=bq_sb[:], in_=bq[:])
            nc.sync.dma_start(out=bk_sb[:], in_=bk[:])
            nc.gpsimd.dma_start(out=ident[:], in_=idin[:])
            nc.gpsimd.dma_start(out=ones_sb[:], in_=onesw[:])

            state = {}

            def alloc_pair(pr):
                st = {"xt": {0: [], 1: []}}
                st["qt"] = qkp.tile([128, S], BF16, tag="qt", name=f"qt_{pr}")
                st["kt"] = qkp.tile([128, S], BF16, tag="kt", name=f"kt_{pr}")
                st["vt"] = vtp.tile([128, S], BF16, tag="vt", name=f"vt_{pr}")
                st["v0"] = vp.tile([128, NKT, DH], BF16, tag="v", name=f"v0_{pr}")
                st["v1"] = vp.tile([128, NKT, DH], BF16, tag="v", name=f"v1_{pr}")
                st["ot"] = otp.tile([128, S], BF16, tag="ot", name=f"ot_{pr}")
                st["dn"] = dnp.tile([65, S], F32, tag="dn", name=f"dn_{pr}")
                for half in range(2):
                    for ci in range(NCH):
                        st["xt"][half].append(
                            xtp.tile([128, S], BF16, tag="xt", name=f"xt_{pr}_{half}_{ci}")
                        )
                state[pr] = st

            def emit_xt_loads(pr):
                # blk-major so block 0 of BOTH halves lands first
                st = state[pr]
                for blk in range(NQB):
                    for half in range(2):
                        b = pr * 2 + half
                        for ci in range(NCH):
                            eng = nc.sync if (ci % 2 == 0) else nc.gpsimd
                            eng.dma_start(
                                out=st["xt"][half][ci][:, bass.ts(blk, 512)],
                                in_=xT[b, bass.ts(ci, 128), bass.ts(blk, 512)],
                            )

            def emit_prep(pr, blk, which, cis, evac):
                """One filler unit: ci-pair matmuls for both halves (+evac)."""
                st = state[pr]
                sl = bass.ts(blk, 512)
                w_sb = {"q": wq_sb, "k": wk_sb, "v": wv_sb}[which]
                key = f"p{which}_{pr}_{blk}"
                if cis[0] == 0:
                    st[key] = psM.tile([128, 512], F32, tag="m", name=key)
                pp = st[key]
                for ci in cis:
                    for half in range(2):
                        nc.tensor.matmul(
                            pp[half * DH:(half + 1) * DH, :],
                            w_sb[:, ci, :], st["xt"][half][ci][:, sl],
                            start=(ci == 0), stop=(ci == NCH - 1),
                            tile_position=(0, half * DH),
                        )
                if evac:
                    if which == "q":
                        nc.vector.tensor_scalar_add(st["qt"][:, sl], pp[:], bq_sb[:])
                    elif which == "k":
                        nc.vector.tensor_scalar_add(st["kt"][:, sl], pp[:], bk_sb[:])
                    else:
                        nc.vector.tensor_copy(st["vt"][:, sl], pp[:])

            def emit_vtr(pr, g):
                """Transpose V^T -> V for key tiles g*4..g*4+3, both halves."""
                st = state[pr]
                pv0 = psM.tile([128, 256], BF16, tag="m", name=f"pvtr0_{pr}_{g}")
                pv1 = psM.tile([128, 256], BF16, tag="m", name=f"pvtr1_{pr}_{g}")
                for j in range(4):
                    nc.tensor.transpose(
                        pv0[:, bass.ts(j, 64)],
                        st["vt"][0:DH, bass.ts(g * 4 + j, 128)],
                        ident[0:DH, 0:DH],
                        tile_position=(0, 0),
                    )
                    nc.tensor.transpose(
                        pv1[:, bass.ts(j, 64)],
                        st["vt"][DH:128, bass.ts(g * 4 + j, 128)],
                        ident[DH:128, DH:128],
                        tile_position=(64, 0),
                    )
                nc.vector.tensor_copy(
                    st["v0"][:, bass.ds(g * 4, 4), :],
                    pv0[:].rearrange("p (k m) -> p k m", m=64),
                )
                nc.vector.tensor_copy(
                    st["v1"][:, bass.ds(g * 4, 4), :],
                    pv1[:].rearrange("p (k m) -> p k m", m=64),
                )

            def emit_op(pr, tt):
                """Out-projection for token tile tt of both batches (pair)."""
                st = state[pr]
                otc = st["ot"]
                pop0 = psM.tile([128, 512], F32, tag="m", name=f"pop0_{pr}_{tt}")
                pop1 = psM.tile([128, 512], F32, tag="m", name=f"pop1_{pr}_{tt}")
                nc.tensor.matmul(
                    pop0[:], otc[0:DH, bass.ts(tt, 128)], wo_sb[0:DH, :],
                    start=True, stop=True, tile_position=(0, 0),
                )
                nc.tensor.matmul(
                    pop1[:], otc[DH:128, bass.ts(tt, 128)], wo_sb[DH:128, :],
                    start=True, stop=True, tile_position=(64, 0),
                )
                so0 = outp.tile([128, 512], BF16, tag="so", name=f"so0_{pr}_{tt}")
                so1 = outp.tile([128, 512], BF16, tag="so", name=f"so1_{pr}_{tt}")
                nc.vector.tensor_copy(so0[:], pop0[:])
                nc.vector.tensor_copy(so1[:], pop1[:])
                b0 = pr * 2
                b1 = pr * 2 + 1
                nc.sync.dma_start(out=out[bass.ds(b0 * S + tt * 128, 128), :], in_=so0[:])
                nc.gpsimd.dma_start(out=out[bass.ds(b1 * S + tt * 128, 128), :], in_=so1[:])

            fq = []   # prep/vtr filler units
            oq = []   # out-projection filler units

            def pump(q, n):
                for _ in range(n):
                    if q:
                        q.pop(0)()

            def emit_attn(pr, qq, f_per_kt=0, o_num=0, o_den=1):
                st = state[pr]
                with nc.named_scope(f"attn_{pr}_{qq}"):
                    sl_q = bass.ts(qq, 512)
                    po = psO.tile([128, 512], F32, tag="po", name=f"po_{pr}_{qq}")
                    psd = psDp.tile([128, 512], F32, tag="pd", name=f"psd_{pr}_{qq}")
                    for kt_i in range(NKT):
                        kt_sl = bass.ts(kt_i, 128)
                        first = kt_i == 0
                        last = kt_i == NKT - 1
                        pst = psA.tile([128, 1024], F32, tag="pst", name=f"pst_{pr}_{qq}_{kt_i}")
                        for hb in range(2):
                            nc.tensor.matmul(
                                pst[:, bass.ts(hb, 512)],
                                st["kt"][hb * DH:(hb + 1) * DH, kt_sl],
                                st["qt"][hb * DH:(hb + 1) * DH, sl_q],
                                start=True, stop=True,
                                tile_position=(hb * DH, 0),
                            )
                        ptt = ptp.tile([128, 1024], BF16, tag="pt", name=f"ptt_{pr}_{qq}_{kt_i}")
                        nc.scalar.activation(ptt[:], pst[:], AF.Exp, scale=0.125)
                        # AV + denominator, paired on disjoint column groups
                        nc.tensor.matmul(
                            po[0:DH, :], st["v0"][:, kt_i, :], ptt[:, 0:512],
                            start=first, stop=last, tile_position=(0, 0),
                        )
                        nc.tensor.matmul(
                            psd[64:65, :], ones_sb[:, 0:1], ptt[:, 512:1024],
                            start=first, stop=last, tile_position=(0, 64),
                        )
                        nc.tensor.matmul(
                            po[DH:128, :], st["v1"][:, kt_i, :], ptt[:, 512:1024],
                            start=first, stop=last, tile_position=(0, 64),
                        )
                        nc.tensor.matmul(
                            psd[0:1, :], ones_sb[:, 0:1], ptt[:, 0:512],
                            start=first, stop=last, tile_position=(0, 0),
                        )
                        pump(fq, f_per_kt)
                        if o_num and (kt_i * o_num) % o_den < o_num:
                            pump(oq, 1)
                    nc.vector.tensor_copy(st["ot"][:, sl_q], po[:])
                    nc.vector.tensor_copy(st["dn"][0:1, sl_q], psd[0:1, :])
                    nc.vector.tensor_copy(st["dn"][64:65, sl_q], psd[64:65, :])

            def emit_den_out(pr):
                st = state[pr]
                nc.gpsimd.dma_start(out=den[bass.ds(pr * 2, 1), :], in_=st["dn"][0:1, :])
                nc.gpsimd.dma_start(out=den[bass.ds(pr * 2 + 1, 1), :], in_=st["dn"][64:65, :])

            # ---------------- emission schedule ----------------
            import functools
            P = functools.partial
            alloc_pair(0)
            alloc_pair(1)
            emit_xt_loads(0)
            emit_xt_loads(1)

            # pair-0 block 0 prep inline (critical path to first exp)
            for w in ("q", "k", "v"):
                emit_prep(0, 0, w, (0, 1), False)
                emit_prep(0, 0, w, (2, 3), True)
            emit_vtr(0, 0)

            # filler inventory, in dependency order
            for blk in (1, 2, 3):
                for w in ("q", "k", "v"):
                    fq.append(P(emit_prep, 0, blk, w, (0, 1), False))
                    fq.append(P(emit_prep, 0, blk, w, (2, 3), True))
                fq.append(P(emit_vtr, 0, blk))
            for blk in range(NQB):
                for w in ("q", "k", "v"):
                    fq.append(P(emit_prep, 1, blk, w, (0, 1), False))
                    fq.append(P(emit_prep, 1, blk, w, (2, 3), True))
                fq.append(P(emit_vtr, 1, blk))

            emit_attn(0, 0, f_per_kt=2)
            emit_attn(0, 1, f_per_kt=2)
            for tt in range(4):
                oq.append(P(emit_op, 0, tt))
            emit_attn(0, 2, f_per_kt=2, o_num=1, o_den=2)
            for tt in range(4, 8):
                oq.append(P(emit_op, 0, tt))
            emit_attn(0, 3, f_per_kt=2, o_num=1, o_den=2)
            emit_den_out(0)
            while fq:
                fq.pop(0)()
            for tt in range(8, 12):
                oq.append(P(emit_op, 0, tt))
            emit_attn(1, 0, o_num=1, o_den=2)
            for tt in range(12, 16):
                oq.append(P(emit_op, 0, tt))
            for tt in range(4):
                oq.append(P(emit_op, 1, tt))
            emit_attn(1, 1, o_num=1, o_den=2)
            for tt in range(4, 8):
                oq.append(P(emit_op, 1, tt))
            emit_attn(1, 2, o_num=1, o_den=2)
            for tt in range(8, 12):
                oq.append(P(emit_op, 1, tt))
            emit_attn(1, 3, o_num=1, o_den=2)
            while oq:
                oq.pop(0)()
            for tt in range(12, 16):
                emit_op(1, tt)
            emit_den_out(1)

    nc.compile()
    return nc


def kernel(x, Wq, bq, Wk, bk, Wv, bv, Wo, bo):
    import ml_dtypes
    x = np.asarray(x, dtype=np.float32)
    xT = np.ascontiguousarray(np.transpose(x, (0, 2, 1))).astype(ml_dtypes.bfloat16)
    Wq = np.asarray(Wq, dtype=np.float32)
    Wk = np.asarray(Wk, dtype=np.float32)
    Wv = np.asarray(Wv, dtype=np.float32)
    Wo = np.asarray(Wo, dtype=np.float32)
    bq = np.asarray(bq, dtype=np.float32)
    bk = np.asarray(bk, dtype=np.float32)
    bv = np.asarray(bv, dtype=np.float32)
    bo = np.asarray(bo, dtype=np.float32)

    if "nc" not in _NC_CACHE:
        _NC_CACHE["nc"] = build_kernel()
    nc = _NC_CACHE["nc"]

    eye = np.eye(128).astype(ml_dtypes.bfloat16)
    ones = np.ones((128, 1), dtype=ml_dtypes.bfloat16)
    in_maps = []
    for c in range(NCORES):
        hs = slice(c * DH, (c + 1) * DH)
        in_maps.append({
            "xT": xT,
            "wq": np.ascontiguousarray(Wq[:, hs]).astype(ml_dtypes.bfloat16),
            "wk": np.ascontiguousarray(Wk[:, hs]).astype(ml_dtypes.bfloat16),
            "wv": np.ascontiguousarray(Wv[:, hs]).astype(ml_dtypes.bfloat16),
            "wo_dup": np.ascontiguousarray(
                np.concatenate([Wo[hs, :], Wo[hs, :]], axis=0)
            ).astype(ml_dtypes.bfloat16),
            "bq": np.ascontiguousarray(np.concatenate([bq[hs], bq[hs]]).reshape(128, 1)),
            "bk": np.ascontiguousarray(np.concatenate([bk[hs], bk[hs]]).reshape(128, 1)),
            "idin": eye,
            "onesw": ones,
        })

    res = run_bass_kernel_spmd(nc, in_maps, list(range(NCORES)))

    acc = np.zeros((B * S, D), dtype=np.float32)
    for c in range(NCORES):
        o = np.asarray(res.results[c]["out"], dtype=np.float32)
        d = np.asarray(res.results[c]["den"], dtype=np.float32).reshape(B * S, 1)
        acc += o / d
    # biases that commute with the head-reduction, applied at gather time
    acc += bo[None, :] + (bv @ Wo)[None, :]
    return acc.reshape(B, S, D)


# revision 9
# speedup vs baseline: 1.4474x; 1.4474x over previous
"""Trainium2 Bass kernel for nn_MultiHeadAttention (B=4, S=2048, D=512, H=8).

Sharding: tensor-parallel over heads — core c owns head c (Dh=64).
Each core computes q/k/v projections for its head slice (full x replicated,
host-pre-transposed to x^T in bf16), attention for its head over all 4
batches, and the partial out-projection O_c @ Wo[c]. The softmax
denominators ride as an extra ones-column of V (row 64 of O^T) through the
out-projection's aug columns; the host divides each partial by its
denominator and sums the 8 partials (the TP all-reduce done at gather
time), adding the biases that commute with that reduction (bo, bv@Wo).

Engine plan: ACT does exclusively exp (128 x [128,1024] instrs, ~1.15us
each — the pacing engine, ~147us floor). PE issues one 512-col matmul per
~216ns slot; row-/col-group disjoint pairs (S^T batch pairs, q/k/v-prep
batch pairs, V-transpose batch pairs) share a slot. Steady-state PE load
per key tile is S^T pair (1 slot) + two 66-wide AV matmuls (2 slots) ~=
756ns < 1146ns exp cadence; projection/transpose/out-projection work is
drip-fed between key tiles from unit queues under a per-tile slot budget so
the PE never delays the next exp. x^T loads are block-major so attention
starts as soon as block 0 lands.
"""
import numpy as np

import concourse.bass as bass
import concourse.mybir as mybir
import concourse.tile as tile
from concourse import bacc
from concourse.bass_utils import run_bass_kernel_spmd

B, S, D = 4, 2048, 512
H, DH = 8, 64
NCORES = 8
F32 = mybir.dt.float32
BF16 = mybir.dt.bfloat16
AF = mybir.ActivationFunctionType

NKT = S // 128          # 16 key tiles per batch
NQB = S // 512          # 4 query blocks per batch
NCH = D // 128          # 4 d_model chunks

_NC_CACHE = {}


def build_kernel():
    nc = bacc.Bacc("TRN2", target_bir_lowering=False, debug=False)

    xT = nc.dram_tensor("xT", [B, D, S], BF16, kind="ExternalInput")
    wq = nc.dram_tensor("wq", [D, DH], BF16, kind="ExternalInput")
    wk = nc.dram_tensor("wk", [D, DH], BF16, kind="ExternalInput")
    wv = nc.dram_tensor("wv", [D, DH], BF16, kind="ExternalInput")
    wo_aug = nc.dram_tensor("wo_aug", [DH + 2, D + 2], BF16, kind="ExternalInput")
    bq = nc.dram_tensor("bq", [128, 1], F32, kind="ExternalInput")
    bk = nc.dram_tensor("bk", [128, 1], F32, kind="ExternalInput")
    idin = nc.dram_tensor("idin", [128, 128], BF16, kind="ExternalInput")
    onesin = nc.dram_tensor("onesin", [128, 16, 2], BF16, kind="ExternalInput")
    out = nc.dram_tensor("out", [B * S, D], BF16, kind="ExternalOutput")
    den = nc.dram_tensor("den", [B * S, 2], F32, kind="ExternalOutput")

    with tile.TileContext(nc) as tc:
        with (
            tc.tile_pool(name="consts", bufs=1) as consts,
            tc.tile_pool(name="xtp", bufs=16) as xtp,
            tc.tile_pool(name="qkp", bufs=2) as qkp,
            tc.tile_pool(name="vtp", bufs=2) as vtp,
            tc.tile_pool(name="vp", bufs=4) as vp,
            tc.tile_pool(name="ptp", bufs=3) as ptp,
            tc.tile_pool(name="otp", bufs=4) as otp,
            tc.tile_pool(name="outp", bufs=6) as outp,
            tc.tile_pool(name="psA", bufs=2, space="PSUM") as psA,   # pst [128,1024] x2 = 4 banks
            tc.tile_pool(name="psO", bufs=2, space="PSUM") as psO,   # po  [66,512]  x2 = 2 banks
            tc.tile_pool(name="psM", bufs=2, space="PSUM") as psM,   # misc [128,512] x2 = 2 banks
        ):
            wq_sb = consts.tile([128, NCH, DH], BF16)
            wk_sb = consts.tile([128, NCH, DH], BF16)
            wv_sb = consts.tile([128, NCH, DH], BF16)
            wo_sb = consts.tile([DH + 2, D + 2], BF16)
            bq_sb = consts.tile([128, 1], F32)
            bk_sb = consts.tile([128, 1], F32)
            ident = consts.tile([128, 128], BF16)
            nc.sync.dma_start(out=wq_sb[:], in_=wq.rearrange("(c p) m -> p c m", p=128))
            nc.sync.dma_start(out=wk_sb[:], in_=wk.rearrange("(c p) m -> p c m", p=128))
            nc.sync.dma_start(out=wv_sb[:], in_=wv.rearrange("(c p) m -> p c m", p=128))
            nc.sync.dma_start(out=wo_sb[:], in_=wo_aug[:])
            nc.sync.dma_start(out=bq_sb[:], in_=bq[:])
            nc.sync.dma_start(out=bk_sb[:], in_=bk[:])
            nc.gpsimd.dma_start(out=ident[:], in_=idin[:])

            state = {}

            def alloc_pair(pr):
                st = {"xt": {0: [], 1: []}, "v": {}, "ot": {}}
                st["qt"] = qkp.tile([128, S], BF16, tag="qt", name=f"qt_{pr}")
                st["kt"] = qkp.tile([128, S], BF16, tag="kt", name=f"kt_{pr}")
                st["vt"] = vtp.tile([128, S], BF16, tag="vt", name=f"vt_{pr}")
                for half in range(2):
                    b = pr * 2 + half
                    st["v"][half] = vp.tile([128, NKT, DH + 2], BF16, tag="v", name=f"v_{b}")
                    st["ot"][half] = otp.tile([DH + 2, S], BF16, tag="ot", name=f"ot_{b}")
                for half in range(2):
                    for ci in range(NCH):
                        st["xt"][half].append(
                            xtp.tile([128, S], BF16, tag="xt", name=f"xt_{pr}_{half}_{ci}")
                        )
                state[pr] = st

            def emit_xt_loads(pr):
                # blk-major so block 0 of BOTH halves lands first
                st = state[pr]
                for blk in range(NQB):
                    for half in range(2):
                        b = pr * 2 + half
                        for ci in range(NCH):
                            eng = nc.sync if (ci % 2 == 0) else nc.gpsimd
                            eng.dma_start(
                                out=st["xt"][half][ci][:, bass.ts(blk, 512)],
                                in_=xT[b, bass.ts(ci, 128), bass.ts(blk, 512)],
                            )

            def emit_ones(pr, half):
                nc.gpsimd.dma_start(
                    out=state[pr]["v"][half][:, :, DH:DH + 2], in_=onesin[:]
                )

            def emit_prep(pr, blk, which, ci):
                """One unit = one d_model chunk for both halves (one PE slot)."""
                st = state[pr]
                sl = bass.ts(blk, 512)
                w_sb = {"q": wq_sb, "k": wk_sb, "v": wv_sb}[which]
                key = f"p{which}_{pr}_{blk}"
                if ci == 0:
                    st[key] = psM.tile([128, 512], F32, tag="m", name=key)
                pp = st[key]
                for half in range(2):
                    nc.tensor.matmul(
                        pp[half * DH:(half + 1) * DH, :],
                        w_sb[:, ci, :], st["xt"][half][ci][:, sl],
                        start=(ci == 0), stop=(ci == NCH - 1),
                        tile_position=(0, half * DH),
                        skip_group_check=True,
                    )
                if ci == NCH - 1:
                    if which == "q":
                        nc.vector.tensor_scalar_add(st["qt"][:, sl], pp[:], bq_sb[:])
                    elif which == "k":
                        nc.vector.tensor_scalar_add(st["kt"][:, sl], pp[:], bk_sb[:])
                    else:
                        nc.vector.tensor_copy(st["vt"][:, sl], pp[:])

            def emit_vtr(pr, g):
                """Transpose V^T -> V_aug for key tiles g*4..g*4+3, both halves."""
                st = state[pr]
                pv0 = psM.tile([128, 256], BF16, tag="m", name=f"pvtr0_{pr}_{g}")
                pv1 = psM.tile([128, 256], BF16, tag="m", name=f"pvtr1_{pr}_{g}")
                for j in range(4):
                    nc.tensor.transpose(
                        pv0[:, bass.ts(j, 64)],
                        st["vt"][0:DH, bass.ts(g * 4 + j, 128)],
                        ident[0:DH, 0:DH],
                        tile_position=(0, 0),
                    )
                    nc.tensor.transpose(
                        pv1[:, bass.ts(j, 64)],
                        st["vt"][DH:128, bass.ts(g * 4 + j, 128)],
                        ident[DH:128, DH:128],
                        tile_position=(64, 0),
                    )
                nc.vector.tensor_copy(
                    st["v"][0][:, bass.ds(g * 4, 4), 0:DH],
                    pv0[:].rearrange("p (k m) -> p k m", m=64),
                )
                nc.vector.tensor_copy(
                    st["v"][1][:, bass.ds(g * 4, 4), 0:DH],
                    pv1[:].rearrange("p (k m) -> p k m", m=64),
                )

            def emit_op(pr, half, tt):
                """Out-projection + denominator extraction for one token tile."""
                st = state[pr]
                b = pr * 2 + half
                otc = st["ot"][half][:, bass.ts(tt, 128)]
                pop = psM.tile([128, 512], F32, tag="m", name=f"pop_{b}_{tt}")
                pos = psM.tile([128, 2], F32, tag="m", name=f"pos_{b}_{tt}")
                nc.tensor.matmul(pop[:], otc, wo_sb[:, 0:D], start=True, stop=True)
                nc.tensor.matmul(pos[:], otc, wo_sb[:, D:D + 2], start=True, stop=True)
                so = outp.tile([128, 512], BF16, tag="so", name=f"so_{b}_{tt}")
                sd = outp.tile([128, 2], F32, tag="sd", name=f"sd_{b}_{tt}")
                nc.vector.tensor_copy(so[:], pop[:])
                nc.vector.tensor_copy(sd[:], pos[:])
                eng = nc.sync if half == 0 else nc.gpsimd
                eng.dma_start(out=out[bass.ds(b * S + tt * 128, 128), :], in_=so[:])
                eng.dma_start(out=den[bass.ds(b * S + tt * 128, 128), :], in_=sd[:])

            # ---- filler unit queues: (weight_in_slots, due_kt, fn) ----
            fq = []
            oq = []

            def pump(q, budget, carry, kt_i):
                budget += carry
                while q and (q[0][1] <= kt_i or q[0][0] <= budget):
                    w, due, fn = q.pop(0)
                    fn()
                    budget -= w
                return min(max(budget, 0.0), 4.0)

            def emit_attn(pr, qq, f_budget=0.0, o_budget=0.0):
                st = state[pr]
                fcarry = ocarry = 0.0
                with nc.named_scope(f"attn_{pr}_{qq}"):
                    sl_q = bass.ts(qq, 512)
                    po = [
                        psO.tile([DH + 2, 512], F32, tag="po", name=f"po{hb}_{pr}_{qq}")
                        for hb in range(2)
                    ]
                    for kt_i in range(NKT):
                        kt_sl = bass.ts(kt_i, 128)
                        pst = psA.tile([128, 1024], F32, tag="pst", name=f"pst_{pr}_{qq}_{kt_i}")
                        for hb in range(2):
                            nc.tensor.matmul(
                                pst[:, bass.ts(hb, 512)],
                                st["kt"][hb * DH:(hb + 1) * DH, kt_sl],
                                st["qt"][hb * DH:(hb + 1) * DH, sl_q],
                                start=True, stop=True,
                                tile_position=(hb * DH, 0),
                            )
                        ptt = ptp.tile([128, 1024], BF16, tag="pt", name=f"ptt_{pr}_{qq}_{kt_i}")
                        nc.scalar.activation(ptt[:], pst[:], AF.Exp, scale=0.125)
                        for hb in range(2):
                            nc.tensor.matmul(
                                po[hb][:],
                                st["v"][hb][:, kt_i, :],
                                ptt[:, bass.ts(hb, 512)],
                                start=(kt_i == 0), stop=(kt_i == NKT - 1),
                            )
                        fcarry = pump(fq, f_budget, fcarry, kt_i)
                        ocarry = pump(oq, o_budget, ocarry, kt_i)
                    for hb in range(2):
                        nc.vector.tensor_copy(st["ot"][hb][:, sl_q], po[hb][:])

            # ---------------- emission schedule ----------------
            import functools
            P = functools.partial
            alloc_pair(0)
            alloc_pair(1)
            emit_xt_loads(0)
            emit_xt_loads(1)
            emit_ones(0, 0)
            emit_ones(0, 1)
            emit_ones(1, 0)
            emit_ones(1, 1)

            # head: pair-0 block-0 prep + first V rung + blk-1 k/v (DMA-paced)
            for w in ("q", "k", "v"):
                for ci in range(NCH):
                    emit_prep(0, 0, w, ci)
            emit_vtr(0, 0)
            for w in ("k", "v"):
                for ci in range(NCH):
                    emit_prep(0, 1, w, ci)

            def prep_units(pr, blk, which, due=99):
                return [(1.0, due, P(emit_prep, pr, blk, which, ci)) for ci in range(NCH)]

            def drain(q):
                while q:
                    q.pop(0)[2]()

            # attn(0,0) fillers: k/v/vtr ladder blks 2-3 + q(0,1) for attn(0,1)
            fq += [(4.0, 1, P(emit_vtr, 0, 1))]
            fq += prep_units(0, 2, "k", 4) + prep_units(0, 2, "v", 4)
            fq += [(4.0, 5, P(emit_vtr, 0, 2))]
            fq += prep_units(0, 3, "k", 8) + prep_units(0, 3, "v", 8)
            fq += [(4.0, 9, P(emit_vtr, 0, 3))]
            fq += prep_units(0, 1, "q")
            emit_attn(0, 0, f_budget=2.6)
            drain(fq)

            fq += prep_units(0, 2, "q")
            fq += prep_units(1, 0, "k") + prep_units(1, 0, "v") + prep_units(1, 0, "q")
            fq += [(4.0, 99, P(emit_vtr, 1, 0))]
            fq += prep_units(1, 1, "k") + prep_units(1, 1, "v")
            emit_attn(0, 1, f_budget=1.8)
            drain(fq)

            fq += prep_units(0, 3, "q")
            fq += prep_units(1, 1, "q")
            fq += [(4.0, 99, P(emit_vtr, 1, 1))]
            fq += prep_units(1, 2, "k") + prep_units(1, 2, "v") + prep_units(1, 2, "q")
            fq += [(4.0, 99, P(emit_vtr, 1, 2))]
            emit_attn(0, 2, f_budget=1.8)
            drain(fq)

            fq += prep_units(1, 3, "k") + prep_units(1, 3, "v") + prep_units(1, 3, "q")
            fq += [(4.0, 99, P(emit_vtr, 1, 3))]
            emit_attn(0, 3, f_budget=1.8)
            drain(fq)

            # out-projections: pr0 + pr1(qq<3) during attn(1,*), pr1 qq3 in tail
            for half in range(2):
                for tt in range(NKT):
                    oq.append((2.0, 99, P(emit_op, 0, half, tt)))
            emit_attn(1, 0, o_budget=1.8)
            for half in range(2):
                for tt in range(4):
                    oq.append((2.0, 99, P(emit_op, 1, half, tt)))
            emit_attn(1, 1, o_budget=1.8)
            for half in range(2):
                for tt in range(4, 8):
                    oq.append((2.0, 99, P(emit_op, 1, half, tt)))
            emit_attn(1, 2, o_budget=1.8)
            for half in range(2):
                for tt in range(8, 12):
                    oq.append((2.0, 99, P(emit_op, 1, half, tt)))
            emit_attn(1, 3, o_budget=1.8)
            drain(oq)
            for half in range(2):
                for tt in range(12, 16):
                    emit_op(1, half, tt)

    nc.compile()
    return nc


def kernel(x, Wq, bq, Wk, bk, Wv, bv, Wo, bo):
    import ml_dtypes
    x = np.asarray(x, dtype=np.float32)
    xT = np.ascontiguousarray(np.transpose(x, (0, 2, 1))).astype(ml_dtypes.bfloat16)
    Wq = np.asarray(Wq, dtype=np.float32)
    Wk = np.asarray(Wk, dtype=np.float32)
    Wv = np.asarray(Wv, dtype=np.float32)
    Wo = np.asarray(Wo, dtype=np.float32)
    bq = np.asarray(bq, dtype=np.float32)
    bk = np.asarray(bk, dtype=np.float32)
    bv = np.asarray(bv, dtype=np.float32)
    bo = np.asarray(bo, dtype=np.float32)

    if "nc" not in _NC_CACHE:
        _NC_CACHE["nc"] = build_kernel()
    nc = _NC_CACHE["nc"]

    eye = np.eye(128).astype(ml_dtypes.bfloat16)
    ones = np.zeros((128, 16, 2), dtype=ml_dtypes.bfloat16)
    ones[:, :, 0] = 1.0
    in_maps = []
    for c in range(NCORES):
        hs = slice(c * DH, (c + 1) * DH)
        wo_a = np.zeros((DH + 2, D + 2), dtype=np.float32)
        wo_a[0:DH, 0:D] = Wo[hs, :]
        wo_a[DH, D] = 1.0
        in_maps.append({
            "xT": xT,
            "wq": np.ascontiguousarray(Wq[:, hs]).astype(ml_dtypes.bfloat16),
            "wk": np.ascontiguousarray(Wk[:, hs]).astype(ml_dtypes.bfloat16),
            "wv": np.ascontiguousarray(Wv[:, hs]).astype(ml_dtypes.bfloat16),
            "wo_aug": wo_a.astype(ml_dtypes.bfloat16),
            "bq": np.ascontiguousarray(np.concatenate([bq[hs], bq[hs]]).reshape(128, 1)),
            "bk": np.ascontiguousarray(np.concatenate([bk[hs], bk[hs]]).reshape(128, 1)),
            "idin": eye,
            "onesin": ones,
        })

    res = run_bass_kernel_spmd(nc, in_maps, list(range(NCORES)))

    acc = np.zeros((B * S, D), dtype=np.float32)
    for c in range(NCORES):
        o = np.asarray(res.results[c]["out"], dtype=np.float32)
        d = np.asarray(res.results[c]["den"], dtype=np.float32)[:, 0:1]
        acc += o / d
    # biases that commute with the head-reduction, applied at gather time
    acc += bo[None, :] + (bv @ Wo)[None, :]
    return acc.reshape(B, S, D)


# revision 10
# speedup vs baseline: 1.5617x; 1.0790x over previous
"""Trainium2 Bass kernel for nn_MultiHeadAttention (B=4, S=2048, D=512, H=8).

Sharding: tensor-parallel over heads — core c owns head c (Dh=64).
Each core computes q/k/v projections for its head slice (full x replicated,
host-pre-transposed to x^T in bf16), attention for its head over all 4
batches, and the partial out-projection O_c @ Wo[c]. The softmax
denominators ride as an extra ones-column of V (row 64 of O^T) through the
out-projection's aug columns; the host divides each partial by its
denominator and sums the 8 partials (the TP all-reduce done at gather
time), adding the biases that commute with that reduction (bo, bv@Wo).

Engine plan: ACT does exclusively exp (128 x [128,1024] instrs, ~1.15us
each — the pacing engine, ~147us floor). PE issues one 512-col matmul per
~216ns slot; row-/col-group disjoint pairs (S^T batch pairs, q/k/v-prep
batch pairs, V-transpose batch pairs) share a slot. Steady-state PE load
per key tile is S^T pair (1 slot) + two 66-wide AV matmuls (2 slots) ~=
756ns < 1146ns exp cadence; projection/transpose/out-projection work is
drip-fed between key tiles from unit queues under a per-tile slot budget so
the PE never delays the next exp. x^T loads are block-major so attention
starts as soon as block 0 lands.
"""
import numpy as np

import concourse.bass as bass
import concourse.mybir as mybir
import concourse.tile as tile
from concourse import bacc
from concourse.bass_utils import run_bass_kernel_spmd

B, S, D = 4, 2048, 512
H, DH = 8, 64
NCORES = 8
F32 = mybir.dt.float32
BF16 = mybir.dt.bfloat16
AF = mybir.ActivationFunctionType

NKT = S // 128          # 16 key tiles per batch
NQB = S // 512          # 4 query blocks per batch
NCH = D // 128          # 4 d_model chunks

_NC_CACHE = {}


def build_kernel():
    nc = bacc.Bacc("TRN2", target_bir_lowering=False, debug=False)

    xT = nc.dram_tensor("xT", [B, D, S], BF16, kind="ExternalInput")
    wq = nc.dram_tensor("wq", [D, DH], BF16, kind="ExternalInput")
    wk = nc.dram_tensor("wk", [D, DH], BF16, kind="ExternalInput")
    wv = nc.dram_tensor("wv", [D, DH], BF16, kind="ExternalInput")
    wo_aug = nc.dram_tensor("wo_aug", [DH + 2, D + 2], BF16, kind="ExternalInput")
    bq = nc.dram_tensor("bq", [128, 1], F32, kind="ExternalInput")
    bk = nc.dram_tensor("bk", [128, 1], F32, kind="ExternalInput")
    idin = nc.dram_tensor("idin", [128, 128], BF16, kind="ExternalInput")
    onesin = nc.dram_tensor("onesin", [128, 16, 2], BF16, kind="ExternalInput")
    out = nc.dram_tensor("out", [B * S, D], BF16, kind="ExternalOutput")
    den = nc.dram_tensor("den", [B, S], F32, kind="ExternalOutput")

    with tile.TileContext(nc) as tc:
        with (
            tc.tile_pool(name="consts", bufs=1) as consts,
            tc.tile_pool(name="xtp", bufs=16) as xtp,
            tc.tile_pool(name="qkp", bufs=2) as qkp,
            tc.tile_pool(name="vtp", bufs=2) as vtp,
            tc.tile_pool(name="vp", bufs=4) as vp,
            tc.tile_pool(name="ptp", bufs=3) as ptp,
            tc.tile_pool(name="otp", bufs=4) as otp,
            tc.tile_pool(name="dnp", bufs=2) as dnp,
            tc.tile_pool(name="outp", bufs=6) as outp,
            tc.tile_pool(name="psA", bufs=2, space="PSUM") as psA,   # pst [128,1024] x2 = 4 banks
            tc.tile_pool(name="psO", bufs=2, space="PSUM") as psO,   # po  [66,512]  x2 = 2 banks
            tc.tile_pool(name="psM", bufs=2, space="PSUM") as psM,   # misc [128,512] x2 = 2 banks
        ):
            wq_sb = consts.tile([128, NCH, DH], BF16)
            wk_sb = consts.tile([128, NCH, DH], BF16)
            wv_sb = consts.tile([128, NCH, DH], BF16)
            wo_sb = consts.tile([DH + 2, D + 2], BF16)
            bq_sb = consts.tile([128, 1], F32)
            bk_sb = consts.tile([128, 1], F32)
            ident = consts.tile([128, 128], BF16)
            nc.sync.dma_start(out=wq_sb[:], in_=wq.rearrange("(c p) m -> p c m", p=128))
            nc.sync.dma_start(out=wk_sb[:], in_=wk.rearrange("(c p) m -> p c m", p=128))
            nc.sync.dma_start(out=wv_sb[:], in_=wv.rearrange("(c p) m -> p c m", p=128))
            nc.sync.dma_start(out=wo_sb[:], in_=wo_aug[:])
            nc.sync.dma_start(out=bq_sb[:], in_=bq[:])
            nc.sync.dma_start(out=bk_sb[:], in_=bk[:])
            nc.gpsimd.dma_start(out=ident[:], in_=idin[:])

            state = {}

            def alloc_pair(pr):
                st = {"xt": {0: [], 1: []}, "v": {}, "ot": {}}
                st["qt"] = qkp.tile([128, S], BF16, tag="qt", name=f"qt_{pr}")
                st["kt"] = qkp.tile([128, S], BF16, tag="kt", name=f"kt_{pr}")
                st["vt"] = vtp.tile([128, S], BF16, tag="vt", name=f"vt_{pr}")
                st["dn"] = dnp.tile([65, 2, S], F32, tag="dn", name=f"dn_{pr}")
                for half in range(2):
                    b = pr * 2 + half
                    st["v"][half] = vp.tile([128, NKT, DH + 2], BF16, tag="v", name=f"v_{b}")
                    st["ot"][half] = otp.tile([DH + 2, S], BF16, tag="ot", name=f"ot_{b}")
                for half in range(2):
                    for ci in range(NCH):
                        st["xt"][half].append(
                            xtp.tile([128, S], BF16, tag="xt", name=f"xt_{pr}_{half}_{ci}")
                        )
                state[pr] = st

            def emit_xt_loads(pr):
                # blk-major so block 0 of BOTH halves lands first
                st = state[pr]
                for blk in range(NQB):
                    for half in range(2):
                        b = pr * 2 + half
                        for ci in range(NCH):
                            eng = nc.sync if (ci % 2 == 0) else nc.gpsimd
                            eng.dma_start(
                                out=st["xt"][half][ci][:, bass.ts(blk, 512)],
                                in_=xT[b, bass.ts(ci, 128), bass.ts(blk, 512)],
                            )

            def emit_ones(pr, half):
                nc.gpsimd.dma_start(
                    out=state[pr]["v"][half][:, :, DH:DH + 2], in_=onesin[:]
                )

            def emit_prep(pr, blk, which, ci):
                """One unit = one d_model chunk for both halves (one PE slot)."""
                st = state[pr]
                sl = bass.ts(blk, 512)
                w_sb = {"q": wq_sb, "k": wk_sb, "v": wv_sb}[which]
                key = f"p{which}_{pr}_{blk}"
                if ci == 0:
                    st[key] = psM.tile([128, 512], F32, tag="m", name=key)
                pp = st[key]
                for half in range(2):
                    nc.tensor.matmul(
                        pp[half * DH:(half + 1) * DH, :],
                        w_sb[:, ci, :], st["xt"][half][ci][:, sl],
                        start=(ci == 0), stop=(ci == NCH - 1),
                        tile_position=(0, half * DH),
                        skip_group_check=True,
                    )
                if ci == NCH - 1:
                    if which == "q":
                        nc.vector.tensor_scalar_add(st["qt"][:, sl], pp[:], bq_sb[:])
                    elif which == "k":
                        nc.vector.tensor_scalar_add(st["kt"][:, sl], pp[:], bk_sb[:])
                    else:
                        nc.vector.tensor_copy(st["vt"][:, sl], pp[:])

            def emit_vtr(pr, g):
                """Transpose V^T -> V_aug for key tiles g*4..g*4+3, both halves."""
                st = state[pr]
                pv0 = psM.tile([128, 256], BF16, tag="m", name=f"pvtr0_{pr}_{g}")
                pv1 = psM.tile([128, 256], BF16, tag="m", name=f"pvtr1_{pr}_{g}")
                for j in range(4):
                    nc.tensor.transpose(
                        pv0[:, bass.ts(j, 64)],
                        st["vt"][0:DH, bass.ts(g * 4 + j, 128)],
                        ident[0:DH, 0:DH],
                        tile_position=(0, 0),
                    )
                    nc.tensor.transpose(
                        pv1[:, bass.ts(j, 64)],
                        st["vt"][DH:128, bass.ts(g * 4 + j, 128)],
                        ident[DH:128, DH:128],
                        tile_position=(64, 0),
                    )
                nc.vector.tensor_copy(
                    st["v"][0][:, bass.ds(g * 4, 4), 0:DH],
                    pv0[:].rearrange("p (k m) -> p k m", m=64),
                )
                nc.vector.tensor_copy(
                    st["v"][1][:, bass.ds(g * 4, 4), 0:DH],
                    pv1[:].rearrange("p (k m) -> p k m", m=64),
                )

            def emit_op(pr, half, tt):
                """Out-projection + denominator extraction for one token tile."""
                st = state[pr]
                b = pr * 2 + half
                otc = st["ot"][half][:, bass.ts(tt, 128)]
                pop = psM.tile([128, 512], F32, tag="m", name=f"pop_{b}_{tt}")
                nc.tensor.matmul(pop[:], otc, wo_sb[:, 0:D], start=True, stop=True)
                so = outp.tile([128, 512], BF16, tag="so", name=f"so_{b}_{tt}")
                nc.vector.tensor_copy(so[:], pop[:])
                eng = nc.sync if half == 0 else nc.gpsimd
                eng.dma_start(out=out[bass.ds(b * S + tt * 128, 128), :], in_=so[:])

            # ---- filler unit queues: (weight_in_slots, due_kt, fn) ----
            fq = []
            oq = []

            def pump(q, budget, carry, kt_i):
                budget += carry
                while q and (q[0][1] <= kt_i or q[0][0] <= budget):
                    w, due, fn = q.pop(0)
                    fn()
                    budget -= w
                return min(max(budget, 0.0), 4.0)

            def emit_attn(pr, qq, f_budget=0.0, o_budget=0.0):
                st = state[pr]
                fcarry = ocarry = 0.0
                with nc.named_scope(f"attn_{pr}_{qq}"):
                    sl_q = bass.ts(qq, 512)
                    po = [
                        psO.tile([DH + 2, 512], F32, tag="po", name=f"po{hb}_{pr}_{qq}")
                        for hb in range(2)
                    ]
                    for kt_i in range(NKT):
                        kt_sl = bass.ts(kt_i, 128)
                        pst = psA.tile([128, 1024], F32, tag="pst", name=f"pst_{pr}_{qq}_{kt_i}")
                        for hb in range(2):
                            nc.tensor.matmul(
                                pst[:, bass.ts(hb, 512)],
                                st["kt"][hb * DH:(hb + 1) * DH, kt_sl],
                                st["qt"][hb * DH:(hb + 1) * DH, sl_q],
                                start=True, stop=True,
                                tile_position=(hb * DH, 0),
                            )
                        ptt = ptp.tile([128, 1024], BF16, tag="pt", name=f"ptt_{pr}_{qq}_{kt_i}")
                        nc.scalar.activation(ptt[:], pst[:], AF.Exp, scale=0.125)
                        for hb in range(2):
                            nc.tensor.matmul(
                                po[hb][:],
                                st["v"][hb][:, kt_i, :],
                                ptt[:, bass.ts(hb, 512)],
                                start=(kt_i == 0), stop=(kt_i == NKT - 1),
                            )
                        fcarry = pump(fq, f_budget, fcarry, kt_i)
                        ocarry = pump(oq, o_budget, ocarry, kt_i)
                    for hb in range(2):
                        nc.vector.tensor_copy(st["ot"][hb][:, sl_q], po[hb][:])
                        nc.vector.tensor_copy(
                            st["dn"][64:65, hb, sl_q], po[hb][64:65, :]
                        )

            # ---------------- emission schedule ----------------
            import functools
            P = functools.partial
            alloc_pair(0)
            alloc_pair(1)
            emit_ones(0, 0)
            emit_ones(0, 1)
            emit_ones(1, 0)
            emit_ones(1, 1)
            emit_xt_loads(0)
            emit_xt_loads(1)

            # PE warmup: ramp the clock while x^T streams in (results unused)
            dummy = psM.tile([128, 256], F32, tag="m", name="dummy")
            for _ in range(14):
                nc.tensor.matmul(
                    dummy[:], ident[:], wq_sb[:].rearrange("p c m -> p (c m)"),
                    start=True, stop=True,
                )

            # head: pair-0 block-0 prep + first V rung + blk-1 k/v (DMA-paced)
            for w in ("q", "k", "v"):
                for ci in range(NCH):
                    emit_prep(0, 0, w, ci)
            emit_vtr(0, 0)

            def prep_units(pr, blk, which, due=99):
                return [(1.0, due, P(emit_prep, pr, blk, which, ci)) for ci in range(NCH)]

            def drain(q):
                while q:
                    q.pop(0)[2]()

            # attn(0,0) fillers: k/v/vtr ladder blks 1-3 + q(0,1) for attn(0,1)
            fq += prep_units(0, 1, "k", 0) + prep_units(0, 1, "v", 1)
            fq += [(4.0, 2, P(emit_vtr, 0, 1))]
            fq += prep_units(0, 2, "k", 4) + prep_units(0, 2, "v", 5)
            fq += [(4.0, 6, P(emit_vtr, 0, 2))]
            fq += prep_units(0, 3, "k", 8) + prep_units(0, 3, "v", 9)
            fq += [(4.0, 10, P(emit_vtr, 0, 3))]
            fq += prep_units(0, 1, "q")
            emit_attn(0, 0, f_budget=2.6)
            drain(fq)

            fq += prep_units(0, 2, "q")
            fq += prep_units(1, 0, "k") + prep_units(1, 0, "v") + prep_units(1, 0, "q")
            fq += [(4.0, 99, P(emit_vtr, 1, 0))]
            fq += prep_units(1, 1, "k") + prep_units(1, 1, "v")
            emit_attn(0, 1, f_budget=1.8)
            drain(fq)

            fq += prep_units(0, 3, "q")
            fq += prep_units(1, 1, "q")
            fq += [(4.0, 99, P(emit_vtr, 1, 1))]
            fq += prep_units(1, 2, "k") + prep_units(1, 2, "v") + prep_units(1, 2, "q")
            fq += [(4.0, 99, P(emit_vtr, 1, 2))]
            emit_attn(0, 2, f_budget=1.8)
            drain(fq)

            fq += prep_units(1, 3, "k") + prep_units(1, 3, "v") + prep_units(1, 3, "q")
            fq += [(4.0, 99, P(emit_vtr, 1, 3))]
            emit_attn(0, 3, f_budget=1.8)
            drain(fq)

            # out-projections: pr0 + pr1(qq<3) during attn(1,*), pr1 qq3 in tail
            for half in range(2):
                for tt in range(NKT):
                    oq.append((1.5, 99, P(emit_op, 0, half, tt)))
            emit_attn(1, 0, o_budget=1.8)
            for half in range(2):
                for tt in range(4):
                    oq.append((1.5, 99, P(emit_op, 1, half, tt)))
            emit_attn(1, 1, o_budget=1.8)
            for half in range(2):
                for tt in range(4, 8):
                    oq.append((1.5, 99, P(emit_op, 1, half, tt)))
            emit_attn(1, 2, o_budget=1.8)
            for half in range(2):
                for tt in range(8, 12):
                    oq.append((1.5, 99, P(emit_op, 1, half, tt)))
            emit_attn(1, 3, o_budget=1.8)
            drain(oq)
            for half in range(2):
                for tt in range(12, 16):
                    emit_op(1, half, tt)
            for pr in range(2):
                for half in range(2):
                    nc.gpsimd.dma_start(
                        out=den[bass.ds(pr * 2 + half, 1), :],
                        in_=state[pr]["dn"][64:65, half, :],
                    )

    nc.compile()
    return nc


def kernel(x, Wq, bq, Wk, bk, Wv, bv, Wo, bo):
    import ml_dtypes
    x = np.asarray(x, dtype=np.float32)
    xT = np.ascontiguousarray(np.transpose(x, (0, 2, 1))).astype(ml_dtypes.bfloat16)
    Wq = np.asarray(Wq, dtype=np.float32)
    Wk = np.asarray(Wk, dtype=np.float32)
    Wv = np.asarray(Wv, dtype=np.float32)
    Wo = np.asarray(Wo, dtype=np.float32)
    bq = np.asarray(bq, dtype=np.float32)
    bk = np.asarray(bk, dtype=np.float32)
    bv = np.asarray(bv, dtype=np.float32)
    bo = np.asarray(bo, dtype=np.float32)

    if "nc" not in _NC_CACHE:
        _NC_CACHE["nc"] = build_kernel()
    nc = _NC_CACHE["nc"]

    eye = np.eye(128).astype(ml_dtypes.bfloat16)
    ones = np.zeros((128, 16, 2), dtype=ml_dtypes.bfloat16)
    ones[:, :, 0] = 1.0
    in_maps = []
    for c in range(NCORES):
        hs = slice(c * DH, (c + 1) * DH)
        wo_a = np.zeros((DH + 2, D + 2), dtype=np.float32)
        wo_a[0:DH, 0:D] = Wo[hs, :]
        wo_a[DH, D] = 1.0
        in_maps.append({
            "xT": xT,
            "wq": np.ascontiguousarray(Wq[:, hs]).astype(ml_dtypes.bfloat16),
            "wk": np.ascontiguousarray(Wk[:, hs]).astype(ml_dtypes.bfloat16),
            "wv": np.ascontiguousarray(Wv[:, hs]).astype(ml_dtypes.bfloat16),
            "wo_aug": wo_a.astype(ml_dtypes.bfloat16),
            "bq": np.ascontiguousarray(np.concatenate([bq[hs], bq[hs]]).reshape(128, 1)),
            "bk": np.ascontiguousarray(np.concatenate([bk[hs], bk[hs]]).reshape(128, 1)),
            "idin": eye,
            "onesin": ones,
        })

    res = run_bass_kernel_spmd(nc, in_maps, list(range(NCORES)))

    acc = np.zeros((B * S, D), dtype=np.float32)
    for c in range(NCORES):
        o = np.asarray(res.results[c]["out"], dtype=np.float32)
        d = np.asarray(res.results[c]["den"], dtype=np.float32).reshape(B * S, 1)
        acc += o / d
    # biases that commute with the head-reduction, applied at gather time
    acc += bo[None, :] + (bv @ Wo)[None, :]
    return acc.reshape(B, S, D)
